# revision 36
# baseline (speedup 1.0000x reference)
"""Causal self-attention (RMS-normed QK, RoPE, GQA) Trainium2 Bass kernel.

Sharding over 8 NeuronCores: 4-way data-parallel over batch x 2-way
tensor-parallel over heads.  Core c handles batch b = c // 2 and head group
g = c % 2 (q heads g*8..g*8+7, kv heads g*2, g*2+1).  Each core produces a
partial output projection; the host sums the two head-group partials per
batch.

v2 design (bf16 data path, fp32 PSUM accumulation everywhere):
  - Host pre-transposes/packs x^T and all weights into the SBUF-native
    [128, ...] partition-major layout, so every DMA is a single contiguous
    [128, N] copy (no on-device transposes of x, no rearrange DMAs).
  - RoPE via a constant +-1 half-swap matrix on the PE (one extra matmul
    per tile) instead of SBUF-SBUF DMAs.
  - RMS-norm: sum-of-squares via ones-matmul to a [1,512] row, sqrt on the
    scalar engine, reciprocal on vector, broadcast back over partitions via
    a K=1 matmul; the normalize multiply runs after RoPE (rope commutes
    with per-column scales), so one vector multiply total.
  - q_gain/sqrt(hd) ride in the per-head `scale` operand of the Exp.
  - Softmax denominators accumulate on the PE (ones-column matmuls into a
    [1,512] PSUM region) instead of vector-engine adds.
  - y^T stays in SBUF in bf16 and feeds the output projection directly as
    the stationary operand (no DRAM spill).
"""

import math

import numpy as np
import ml_dtypes

import concourse.bass as bass
import concourse.mybir as mybir
import concourse.tile as tile
from concourse import bacc, bass_utils
from concourse.masks import make_identity

F32 = mybir.dt.float32
F32R = mybir.dt.float32r
BF16 = mybir.dt.bfloat16
BF16NP = ml_dtypes.bfloat16

HEAD_DIM = 128
N_HEADS = 16
N_KV_HEADS = 4
ROPE_BASE = 10000.0
TRAIN_SEQ_LEN = 1024

B, D = 4, 2048
H_LOC = 8  # q heads per core
KV_LOC = 2  # kv heads per core
EC = D // 128  # contraction chunks
EPS = float(np.finfo(np.float32).eps)
INV_SQRT_HD = 1.0 / math.sqrt(HEAD_DIM)
AF = mybir.ActivationFunctionType


def _rope_tables(T):
    rd = HEAD_DIM
    base = ROPE_BASE
    if T > TRAIN_SEQ_LEN:
        scale = T / TRAIN_SEQ_LEN
        base = base * scale ** (rd / (rd - 2))
    inv_freq = 1.0 / base ** (np.arange(0, rd, 2, dtype=np.float32) / rd)
    freqs = np.outer(np.arange(T, dtype=np.float32), inv_freq)
    return np.cos(freqs).astype(np.float32), np.sin(freqs).astype(np.float32)


def _blob_layout(T):
    """(name, n_bf16_elements) regions of the packed input blob."""
    return [
        ("xt", 128 * EC * T),
        ("qwt", 128 * H_LOC * EC * 128),
        ("kwt", 128 * KV_LOC * EC * 128),
        ("vwt", 128 * KV_LOC * EC * 128),
        ("owt", 128 * H_LOC * D),
        ("cos2", 128 * T),
        ("sin2", 128 * T),
        ("swapm", 128 * 128),
        ("gains", 2 * H_LOC),  # H_LOC f32 values as raw bf16 pairs
    ]


def build_program(T=2048, phases=(1, 2, 3)):
    """Build the per-core Bass program. T must be a multiple of 512."""
    assert T % 512 == 0
    NT = T // 128  # 128-wide t tiles
    NTB = T // 512  # projection column chunks
    NIB = T // 512  # attention i blocks

    nc = bacc.Bacc("TRN2", target_bir_lowering=False, debug=False, num_devices=8)

    # All inputs live in ONE flat bf16 blob (a single runtime buffer per
    # call is measurably cheaper to dispatch through the runtime than ten).
    sizes = _blob_layout(T)
    total = sum(n for _, n in sizes)
    blob_d = nc.dram_tensor("blob", [total], BF16, kind="ExternalInput").ap()
    regions = {}
    off = 0
    for name, n in sizes:
        regions[name] = blob_d[off:off + n]
        off += n

    def blob_ap(name, free_shape):
        """region as a [128, *free_shape] partition-major AP (C order)."""
        r = regions[name]
        dims = list(free_shape)
        strides = []
        s = 1
        for d in reversed(dims):
            strides.append((s, d))
            s *= d
        strides.reverse()
        ap = [[s, 128]] + [[st, d] for st, d in strides]
        return bass.AP(tensor=r.tensor, offset=r.offset, ap=ap)

    xt_d = blob_ap("xt", (EC, T))
    qwt_d = blob_ap("qwt", (H_LOC, EC, 128))
    kwt_d = blob_ap("kwt", (KV_LOC, EC, 128))
    vwt_d = blob_ap("vwt", (KV_LOC, EC, 128))
    owt_d = blob_ap("owt", (H_LOC, D))
    cos_d = blob_ap("cos2", (T,))
    sin_d = blob_ap("sin2", (T,))
    swap_d = blob_ap("swapm", (128,))
    g_r = regions["gains"]  # [2*H_LOC] bf16 holding H_LOC f32 values
    gains_d = bass.AP(tensor=g_r.tensor, offset=g_r.offset,
                      ap=[[0, 128], [1, 2 * H_LOC]])
    out_d = nc.dram_tensor("out", [T, D], F32, kind="ExternalOutput").ap()

    with tile.TileContext(nc) as tc:
        with (
            tc.tile_pool(name="const", bufs=1) as const_p,
            tc.tile_pool(name="pers", bufs=1) as pers_p,
        ):
            ident = const_p.tile([128, 128], BF16)
            make_identity(nc, ident)
            ones_p = const_p.tile([128, 1], BF16)  # lhsT for partition-sum
            nc.vector.memset(ones_p, 1.0)
            ones_c = const_p.tile([1, 128], F32R)  # lhsT for K=1 broadcast
            nc.vector.memset(ones_c.bitcast(F32), 1.0)
            eps1_sb = const_p.tile([1, 1], F32)
            nc.vector.memset(eps1_sb, EPS)
            swap_sb = const_p.tile([128, 128], BF16)
            nc.sync.dma_start(swap_sb, swap_d)
            cos_sb = const_p.tile([128, T], BF16)
            nc.sync.dma_start(cos_sb, cos_d)
            sin_sb = const_p.tile([128, T], BF16)
            nc.sync.dma_start(sin_sb, sin_d)
            # gains arrive as raw f32 bits inside the bf16 blob; broadcast
            # them to all partitions via a stride-0 DMA, then scale.
            graw_sb = const_p.tile([128, 2 * H_LOC], BF16)
            nc.sync.dma_start(graw_sb, gains_d)
            gscl_sb = const_p.tile([128, H_LOC], F32)
            nc.scalar.mul(gscl_sb, graw_sb.bitcast(F32), INV_SQRT_HD)
            # causal 0/1 masks for the 4 diagonal j-tiles of an i-block:
            # mask[p, v, c] = 1 iff c >= 128*v + p
            mask_sb = const_p.tile([128, 4, 512], BF16)
            nc.vector.memset(mask_sb, 1.0)
            nc.gpsimd.affine_select(
                out=mask_sb, in_=mask_sb,
                compare_op=mybir.AluOpType.is_ge, fill=0.0,
                base=0, channel_multiplier=-1,
                pattern=[[-128, 4], [1, 512]])

            qT = pers_p.tile([128, H_LOC, T], BF16)
            kT = pers_p.tile([128, KV_LOC, T], BF16)
            v_sb = pers_p.tile([128, NT, KV_LOC * 128], BF16)
            yT = pers_p.tile([128, H_LOC, T], BF16)

            # ---------------- Phase 1: projections -----------------------
            with (
                tc.tile_pool(name="p1xt", bufs=1) as xt_p,
                tc.tile_pool(name="p1w", bufs=2) as w_p,
                tc.tile_pool(name="p1wk", bufs=2) as wk_p,
                tc.tile_pool(name="p1row", bufs=2) as row_p,
                tc.tile_pool(name="p1psh", bufs=3, space="PSUM") as ps_h,
                tc.tile_pool(name="p1psrow", bufs=2, space="PSUM") as ps_row,
                tc.tile_pool(name="p1psq", bufs=3, space="PSUM") as ps_q,
            ):
                xt = xt_p.tile([128, EC, T], BF16)
                for tb in range(NTB):
                    tsl = slice(tb * 512, (tb + 1) * 512)
                    nc.gpsimd.dma_start(xt[:, :, tsl], xt_d[:, :, tsl])

                def load_w(w_dram, idx):
                    wt = w_p.tile([128, EC, 128], BF16, tag="w")
                    nc.sync.dma_start(wt, w_dram[:, idx, :, :])
                    return wt

                def project_chunk(wt, tsl):
                    h_ps = ps_h.tile([128, 512], F32, tag="hps")
                    for e in range(EC):
                        nc.tensor.matmul(h_ps, wt[:, e, :], xt[:, e, tsl],
                                         start=(e == 0), stop=(e == EC - 1))
                    return h_ps

                def norm_rope_chunk(h_ps, tsl, dst):
                    """dst = rms_norm+rope of the raw projection chunk."""
                    # sum of squares over the head dim (partitions)
                    sq = wk_p.tile([128, 512], BF16, tag="sq")
                    nc.scalar.square(sq, h_ps)
                    ssq_ps = ps_row.tile([1, 512], F32, tag="row")
                    nc.tensor.matmul(ssq_ps, ones_p, sq, start=True, stop=True)
                    rms_row = row_p.tile([1, 512], F32, tag="rmsr")
                    nc.scalar.activation(rms_row, ssq_ps, AF.Sqrt,
                                         bias=eps1_sb, scale=1.0 / 128.0)
                    rinv_row = row_p.tile([1, 512], F32R, tag="rinvr")
                    with nc.allow_low_precision(reason="f32r matmul operand"):
                        nc.vector.reciprocal(rinv_row, rms_row)
                    rinv_ps = ps_row.tile([128, 512], F32, tag="row")
                    nc.tensor.matmul(rinv_ps, ones_c, rinv_row,
                                     start=True, stop=True)
                    # rope: rot = x*cos + (S@x)*sin, then * rinv (commutes)
                    x_sb = wk_p.tile([128, 512], BF16, tag="xsb")
                    nc.scalar.copy(x_sb, h_ps)
                    qs_ps = ps_q.tile([128, 512], F32, tag="qsps")
                    nc.tensor.matmul(qs_ps, swap_sb, x_sb, start=True, stop=True)
                    rc = wk_p.tile([128, 512], BF16, tag="rc")
                    nc.vector.tensor_mul(rc, x_sb, cos_sb[:, tsl])
                    qsw = wk_p.tile([128, 512], BF16, tag="qsw")
                    nc.vector.tensor_mul(qsw, qs_ps, sin_sb[:, tsl])
                    qr = wk_p.tile([128, 512], BF16, tag="qr")
                    nc.vector.tensor_add(qr, rc, qsw)
                    nc.vector.tensor_mul(dst, qr, rinv_ps)

                for kv in range(KV_LOC):
                    wt = load_w(kwt_d, kv)
                    for tb in range(NTB):
                        tsl = slice(tb * 512, (tb + 1) * 512)
                        h_ps = project_chunk(wt, tsl)
                        norm_rope_chunk(h_ps, tsl, kT[:, kv, tsl])

                    # V head: plain projection, transpose to natural layout.
                    wtv = load_w(vwt_d, kv)
                    for tb in range(NTB):
                        tsl = slice(tb * 512, (tb + 1) * 512)
                        v_ps = project_chunk(wtv, tsl)
                        vt = wk_p.tile([128, 512], BF16, tag="vt")
                        nc.vector.tensor_copy(vt, v_ps)
                        for tt in range(4):
                            pst = ps_q.tile([128, 128], BF16, tag="qsps")
                            nc.tensor.transpose(
                                pst, vt[:, tt * 128:(tt + 1) * 128], ident)
                            nc.vector.tensor_copy(
                                v_sb[:, tb * 4 + tt,
                                     kv * 128:(kv + 1) * 128], pst)

                for h in range(H_LOC):
                    wt = load_w(qwt_d, h)
                    for tb in range(NTB):
                        tsl = slice(tb * 512, (tb + 1) * 512)
                        h_ps = project_chunk(wt, tsl)
                        norm_rope_chunk(h_ps, tsl, qT[:, h, tsl])

            # ---------------- Phase 2: attention --------------------------
            with (
                tc.tile_pool(name="p2pt", bufs=4) as pt_p,
                tc.tile_pool(name="p2y", bufs=2) as ystg_p,
                tc.tile_pool(name="p2row", bufs=2) as row2_p,
                tc.tile_pool(name="p2pss", bufs=4, space="PSUM") as ps_s,
                tc.tile_pool(name="p2psy", bufs=2, space="PSUM") as ps_y,
                tc.tile_pool(name="p2psl", bufs=2, space="PSUM") as ps_l,
            ):
                for h in range(H_LOC if 2 in phases else 0):
                    kv = h // (N_HEADS // N_KV_HEADS)
                    for ib in range(NIB):
                        jmax = 4 * ib + 3
                        isl = slice(ib * 512, (ib + 1) * 512)
                        y_ps = ps_y.tile([128, 512], F32, tag="y")
                        l_ps = ps_l.tile([1, 512], F32, tag="l")
                        for jt in range(jmax + 1):
                            s_ps = ps_s.tile([128, 512], F32, tag="s")
                            nc.tensor.matmul(
                                s_ps,
                                kT[:, kv, jt * 128:(jt + 1) * 128],
                                qT[:, h, isl], start=True, stop=True)
                            pt = pt_p.tile([128, 512], BF16, tag="pt")
                            nc.scalar.activation(
                                pt, s_ps, AF.Exp, scale=gscl_sb[:, h:h + 1])
                            if jt >= 4 * ib:  # diagonal j-tile: causal mask
                                ptm = pt_p.tile([128, 512], BF16, tag="ptm")
                                nc.vector.tensor_mul(
                                    ptm, pt, mask_sb[:, jt - 4 * ib, :])
                                pt = ptm
                            nc.tensor.matmul(
                                l_ps, ones_p, pt,
                                start=(jt == 0), stop=(jt == jmax))
                            nc.tensor.matmul(
                                y_ps,
                                v_sb[:, jt, kv * 128:(kv + 1) * 128],
                                pt,
                                start=(jt == 0), stop=(jt == jmax))
                        lrow = row2_p.tile([1, 512], F32R, tag="lr")
                        with nc.allow_low_precision(reason="f32r matmul operand"):
                            nc.vector.reciprocal(lrow, l_ps)
                        linv_ps = ps_l.tile([128, 512], F32, tag="l")
                        nc.tensor.matmul(linv_ps, ones_c, lrow,
                                         start=True, stop=True)
                        linv_sb = ystg_p.tile([128, 512], BF16, tag="linv")
                        nc.scalar.copy(linv_sb, linv_ps)
                        nc.vector.tensor_mul(yT[:, h, isl], y_ps, linv_sb)

            # ---------------- Phase 3: output projection ------------------
            with (
                tc.tile_pool(name="p3ow", bufs=1) as ow_p,
                tc.tile_pool(name="p3o", bufs=2) as ostg_p,
                tc.tile_pool(name="p3ps", bufs=4, space="PSUM") as ps_o,
            ):
                ow_sb = ow_p.tile([128, H_LOC, D], BF16)
                nc.sync.dma_start(ow_sb, owt_d)
                for it in range(NT if 3 in phases else 0):
                    o_sb = ostg_p.tile([128, D], F32, tag="osb")
                    for db in range(D // 512):
                        o_ps = ps_o.tile([128, 512], F32, tag="o")
                        for h in range(H_LOC):
                            nc.tensor.matmul(
                                o_ps,
                                yT[:, h, it * 128:(it + 1) * 128],
                                ow_sb[:, h, db * 512:(db + 1) * 512],
                                start=(h == 0), stop=(h == H_LOC - 1))
                        nc.scalar.copy(o_sb[:, db * 512:(db + 1) * 512], o_ps)
                    nc.sync.dma_start(out_d[it * 128:(it + 1) * 128, :], o_sb)

    nc.compile()
    return nc


def _pack_weight(w):
    """w [ncols, D]: returns [128, nh, EC, 128] with
    out[p, h, e, c] = w[h*128 + c, e*128 + p]."""
    nh = w.shape[0] // 128
    return np.ascontiguousarray(
        w.reshape(nh, 128, EC, 128).transpose(3, 0, 2, 1)).astype(BF16NP)


def make_in_maps(x, q_w, k_w, v_w, out_w, q_gain, T):
    cos, sin = _rope_tables(T)
    cosT = np.ascontiguousarray(cos.T)  # [64, T]
    sinT = np.ascontiguousarray(sin.T)
    cos2 = np.concatenate([cosT, cosT], axis=0).astype(BF16NP)  # [128, T]
    sin2 = np.concatenate([sinT, sinT], axis=0).astype(BF16NP)
    S = np.zeros((128, 128), dtype=np.float32)
    S[np.arange(64), 64 + np.arange(64)] = 1.0
    S[64 + np.arange(64), np.arange(64)] = -1.0
    swapm = np.ascontiguousarray(S.T).astype(BF16NP)

    layout = _blob_layout(T)
    in_maps = []
    for c in range(8):
        b, g = c // 2, c % 2
        # xt[p, e, t] = x[b][t, e*128+p]
        xtp = np.ascontiguousarray(
            x[b].T.reshape(EC, 128, T).transpose(1, 0, 2)).astype(BF16NP)
        # ow[p, h, d] = out_w[d, g*1024 + h*128 + p]
        owp = np.ascontiguousarray(
            out_w[:, g * 1024:(g + 1) * 1024].T
            .reshape(H_LOC, 128, D).transpose(1, 0, 2)).astype(BF16NP)
        parts = {
            "xt": xtp,
            "qwt": _pack_weight(q_w[g * 1024:(g + 1) * 1024, :]),
            "kwt": _pack_weight(k_w[g * 256:(g + 1) * 256, :]),
            "vwt": _pack_weight(v_w[g * 256:(g + 1) * 256, :]),
            "owt": owp,
            "cos2": cos2,
            "sin2": sin2,
            "swapm": swapm,
            "gains": np.ascontiguousarray(
                q_gain[g * H_LOC:(g + 1) * H_LOC]).astype(np.float32)
                .view(BF16NP),
        }
        blob = np.concatenate(
            [np.asarray(parts[name]).reshape(-1) for name, _ in layout])
        for (name, n), arr in zip(layout, [parts[n] for n, _ in layout]):
            assert np.asarray(arr).size == n, (name, np.asarray(arr).size, n)
        in_maps.append({"blob": blob})
    return in_maps


def kernel(x, q_w, k_w, v_w, out_w, q_gain, _trace=False, _trace_cores=None):
    x = np.asarray(x, dtype=np.float32)
    q_w = np.asarray(q_w, dtype=np.float32)
    k_w = np.asarray(k_w, dtype=np.float32)
    v_w = np.asarray(v_w, dtype=np.float32)
    out_w = np.asarray(out_w, dtype=np.float32)
    q_gain = np.asarray(q_gain, dtype=np.float32)
    T = x.shape[1]

    nc = build_program(T)
    in_maps = make_in_maps(x, q_w, k_w, v_w, out_w, q_gain, T)
    res = bass_utils.run_bass_kernel_spmd(
        nc, in_maps, core_ids=list(range(8)),
        trace=_trace, trace_cores=_trace_cores)
    outs = [r["out"] for r in res.results]
    full = np.stack([outs[2 * b] + outs[2 * b + 1] for b in range(B)])
    if _trace:
        return full.astype(np.float32), res
    return full.astype(np.float32)


# revision 39
# speedup vs baseline: 1.0440x; 1.0440x over previous
"""Causal self-attention (RMS-normed QK, RoPE, GQA) Trainium2 Bass kernel.

Sharding over 8 NeuronCores: 4-way data-parallel over batch x 2-way
tensor-parallel over heads.  Core c handles batch b = c // 2 and head group
g = c % 2 (q heads g*8..g*8+7, kv heads g*2, g*2+1).  Each core produces a
partial output projection; the host sums the two head-group partials per
batch.

v2 design (bf16 data path, fp32 PSUM accumulation everywhere):
  - Host pre-transposes/packs x^T and all weights into the SBUF-native
    [128, ...] partition-major layout, so every DMA is a single contiguous
    [128, N] copy (no on-device transposes of x, no rearrange DMAs).
  - RoPE via a constant +-1 half-swap matrix on the PE (one extra matmul
    per tile) instead of SBUF-SBUF DMAs.
  - RMS-norm: sum-of-squares via ones-matmul to a [1,512] row, sqrt on the
    scalar engine, reciprocal on vector, broadcast back over partitions via
    a K=1 matmul; the normalize multiply runs after RoPE (rope commutes
    with per-column scales), so one vector multiply total.
  - q_gain/sqrt(hd) ride in the per-head `scale` operand of the Exp.
  - Softmax denominators accumulate on the PE (ones-column matmuls into a
    [1,512] PSUM region) instead of vector-engine adds.
  - y^T stays in SBUF in bf16 and feeds the output projection directly as
    the stationary operand (no DRAM spill).
"""

import math

import numpy as np
import ml_dtypes

import concourse.bass as bass
import concourse.mybir as mybir
import concourse.tile as tile
from concourse import bacc, bass_utils
from concourse.masks import make_identity

F32 = mybir.dt.float32
F32R = mybir.dt.float32r
BF16 = mybir.dt.bfloat16
BF16NP = ml_dtypes.bfloat16

HEAD_DIM = 128
N_HEADS = 16
N_KV_HEADS = 4
ROPE_BASE = 10000.0
TRAIN_SEQ_LEN = 1024

B, D = 4, 2048
H_LOC = 8  # q heads per core
KV_LOC = 2  # kv heads per core
EC = D // 128  # contraction chunks
EPS = float(np.finfo(np.float32).eps)
INV_SQRT_HD = 1.0 / math.sqrt(HEAD_DIM)
AF = mybir.ActivationFunctionType


def _rope_tables(T):
    rd = HEAD_DIM
    base = ROPE_BASE
    if T > TRAIN_SEQ_LEN:
        scale = T / TRAIN_SEQ_LEN
        base = base * scale ** (rd / (rd - 2))
    inv_freq = 1.0 / base ** (np.arange(0, rd, 2, dtype=np.float32) / rd)
    freqs = np.outer(np.arange(T, dtype=np.float32), inv_freq)
    return np.cos(freqs).astype(np.float32), np.sin(freqs).astype(np.float32)


def _blob_layout(T):
    """(name, n_bf16_elements) regions of the packed input blob."""
    return [
        ("xt", 128 * EC * T),
        ("qwt", 128 * H_LOC * EC * 128),
        ("kwt", 128 * KV_LOC * EC * 128),
        ("vwt", 128 * KV_LOC * EC * 128),
        ("owt", 128 * H_LOC * D),
        ("cos2", 128 * T),
        ("sin2", 128 * T),
        ("swapm", 128 * 128),
        ("gains", 2 * H_LOC),  # H_LOC f32 values as raw bf16 pairs
    ]


def build_program(T=2048, phases=(1, 2, 3)):
    """Build the per-core Bass program. T must be a multiple of 512."""
    assert T % 512 == 0
    NT = T // 128  # 128-wide t tiles
    NTB = T // 512  # projection column chunks
    NIB = T // 512  # attention i blocks

    nc = bacc.Bacc("TRN2", target_bir_lowering=False, debug=False, num_devices=8)

    # All inputs live in ONE flat bf16 blob (a single runtime buffer per
    # call is measurably cheaper to dispatch through the runtime than ten).
    sizes = _blob_layout(T)
    total = sum(n for _, n in sizes)
    blob_d = nc.dram_tensor("blob", [total], BF16, kind="ExternalInput").ap()
    regions = {}
    off = 0
    for name, n in sizes:
        regions[name] = blob_d[off:off + n]
        off += n

    def blob_ap(name, free_shape):
        """region as a [128, *free_shape] partition-major AP (C order)."""
        r = regions[name]
        dims = list(free_shape)
        strides = []
        s = 1
        for d in reversed(dims):
            strides.append((s, d))
            s *= d
        strides.reverse()
        ap = [[s, 128]] + [[st, d] for st, d in strides]
        return bass.AP(tensor=r.tensor, offset=r.offset, ap=ap)

    xt_d = blob_ap("xt", (EC, T))
    qwt_d = blob_ap("qwt", (H_LOC, EC, 128))
    kwt_d = blob_ap("kwt", (KV_LOC, EC, 128))
    vwt_d = blob_ap("vwt", (KV_LOC, EC, 128))
    owt_d = blob_ap("owt", (H_LOC, D))
    cos_d = blob_ap("cos2", (T,))
    sin_d = blob_ap("sin2", (T,))
    swap_d = blob_ap("swapm", (128,))
    g_r = regions["gains"]  # [2*H_LOC] bf16 holding H_LOC f32 values
    gains_d = bass.AP(tensor=g_r.tensor, offset=g_r.offset,
                      ap=[[0, 128], [1, 2 * H_LOC]])
    out_d = nc.dram_tensor("out", [T, D], F32, kind="ExternalOutput").ap()

    with tile.TileContext(nc) as tc:
        with (
            tc.tile_pool(name="const", bufs=1) as const_p,
            tc.tile_pool(name="pers", bufs=1) as pers_p,
        ):
            ident = const_p.tile([128, 128], BF16)
            make_identity(nc, ident)
            ones_p = const_p.tile([128, 1], BF16)  # lhsT for partition-sum
            nc.vector.memset(ones_p, 1.0)
            ones_c = const_p.tile([1, 128], F32R)  # lhsT for K=1 broadcast
            nc.vector.memset(ones_c.bitcast(F32), 1.0)
            eps1_sb = const_p.tile([1, 1], F32)
            nc.vector.memset(eps1_sb, EPS)
            swap_sb = const_p.tile([128, 128], BF16)
            nc.sync.dma_start(swap_sb, swap_d)
            cos_sb = const_p.tile([128, T], BF16)
            nc.sync.dma_start(cos_sb, cos_d)
            sin_sb = const_p.tile([128, T], BF16)
            nc.sync.dma_start(sin_sb, sin_d)
            # gains arrive as raw f32 bits inside the bf16 blob; broadcast
            # them to all partitions via a stride-0 DMA, then scale.
            graw_sb = const_p.tile([128, 2 * H_LOC], BF16)
            nc.sync.dma_start(graw_sb, gains_d)
            gscl_sb = const_p.tile([128, H_LOC], F32)
            nc.scalar.mul(gscl_sb, graw_sb.bitcast(F32), INV_SQRT_HD)
            # causal 0/1 masks for the 4 diagonal j-tiles of an i-block:
            # mask[p, v, c] = 1 iff c >= 128*v + p
            mask_sb = const_p.tile([128, 4, 512], BF16)
            nc.vector.memset(mask_sb, 1.0)
            nc.gpsimd.affine_select(
                out=mask_sb, in_=mask_sb,
                compare_op=mybir.AluOpType.is_ge, fill=0.0,
                base=0, channel_multiplier=-1,
                pattern=[[-128, 4], [1, 512]])

            qT = pers_p.tile([128, H_LOC, T], BF16)
            kT = pers_p.tile([128, KV_LOC, T], BF16)
            v_sb = pers_p.tile([128, NT, KV_LOC * 128], BF16)
            yT = pers_p.tile([128, H_LOC, T], BF16)

            # ---------------- Phase 1: projections -----------------------
            with (
                tc.tile_pool(name="p1xt", bufs=1) as xt_p,
                tc.tile_pool(name="p1w", bufs=2) as w_p,
                tc.tile_pool(name="p1wk", bufs=2) as wk_p,
                tc.tile_pool(name="p1row", bufs=2) as row_p,
                tc.tile_pool(name="p1psh", bufs=3, space="PSUM") as ps_h,
                tc.tile_pool(name="p1psrow", bufs=2, space="PSUM") as ps_row,
                tc.tile_pool(name="p1psq", bufs=3, space="PSUM") as ps_q,
            ):
                xt = xt_p.tile([128, EC, T], BF16)
                for tb in range(NTB):
                    tsl = slice(tb * 512, (tb + 1) * 512)
                    nc.gpsimd.dma_start(xt[:, :, tsl], xt_d[:, :, tsl])

                def load_w(w_dram, idx):
                    wt = w_p.tile([128, EC, 128], BF16, tag="w")
                    nc.sync.dma_start(wt, w_dram[:, idx, :, :])
                    return wt

                def project_chunk(wt, tsl):
                    h_ps = ps_h.tile([128, 512], F32, tag="hps")
                    for e in range(EC):
                        nc.tensor.matmul(h_ps, wt[:, e, :], xt[:, e, tsl],
                                         start=(e == 0), stop=(e == EC - 1))
                    return h_ps

                def norm_rope_chunk(h_ps, tsl, dst):
                    """dst = rms_norm+rope of the raw projection chunk."""
                    # sum of squares over the head dim (partitions)
                    sq = wk_p.tile([128, 512], BF16, tag="sq")
                    nc.scalar.square(sq, h_ps)
                    ssq_ps = ps_row.tile([1, 512], F32, tag="row")
                    nc.tensor.matmul(ssq_ps, ones_p, sq, start=True, stop=True)
                    rms_row = row_p.tile([1, 512], F32, tag="rmsr")
                    nc.scalar.activation(rms_row, ssq_ps, AF.Sqrt,
                                         bias=eps1_sb, scale=1.0 / 128.0)
                    rinv_row = row_p.tile([1, 512], F32R, tag="rinvr")
                    with nc.allow_low_precision(reason="f32r matmul operand"):
                        nc.vector.reciprocal(rinv_row, rms_row)
                    rinv_ps = ps_row.tile([128, 512], F32, tag="row")
                    nc.tensor.matmul(rinv_ps, ones_c, rinv_row,
                                     start=True, stop=True)
                    # rope: rot = x*cos + (S@x)*sin, then * rinv (commutes)
                    x_sb = wk_p.tile([128, 512], BF16, tag="xsb")
                    nc.scalar.copy(x_sb, h_ps)
                    qs_ps = ps_q.tile([128, 512], F32, tag="qsps")
                    nc.tensor.matmul(qs_ps, swap_sb, x_sb, start=True, stop=True)
                    rc = wk_p.tile([128, 512], BF16, tag="rc")
                    nc.vector.tensor_mul(rc, x_sb, cos_sb[:, tsl])
                    qsw = wk_p.tile([128, 512], BF16, tag="qsw")
                    nc.vector.tensor_mul(qsw, qs_ps, sin_sb[:, tsl])
                    qr = wk_p.tile([128, 512], BF16, tag="qr")
                    nc.vector.tensor_add(qr, rc, qsw)
                    nc.vector.tensor_mul(dst, qr, rinv_ps)

                for kv in range(KV_LOC):
                    wt = load_w(kwt_d, kv)
                    for tb in range(NTB):
                        tsl = slice(tb * 512, (tb + 1) * 512)
                        h_ps = project_chunk(wt, tsl)
                        norm_rope_chunk(h_ps, tsl, kT[:, kv, tsl])

                    # V head: plain projection, transpose to natural layout.
                    wtv = load_w(vwt_d, kv)
                    for tb in range(NTB):
                        tsl = slice(tb * 512, (tb + 1) * 512)
                        v_ps = project_chunk(wtv, tsl)
                        vt = wk_p.tile([128, 512], BF16, tag="vt")
                        nc.vector.tensor_copy(vt, v_ps)
                        for tt in range(4):
                            pst = ps_q.tile([128, 128], BF16, tag="qsps")
                            nc.tensor.transpose(
                                pst, vt[:, tt * 128:(tt + 1) * 128], ident)
                            nc.vector.tensor_copy(
                                v_sb[:, tb * 4 + tt,
                                     kv * 128:(kv + 1) * 128], pst)

                for h in range(H_LOC):
                    wt = load_w(qwt_d, h)
                    for tb in range(NTB):
                        tsl = slice(tb * 512, (tb + 1) * 512)
                        h_ps = project_chunk(wt, tsl)
                        norm_rope_chunk(h_ps, tsl, qT[:, h, tsl])

            # ---------------- Phase 2: attention --------------------------
            with (
                tc.tile_pool(name="p2pt", bufs=4) as pt_p,
                tc.tile_pool(name="p2y", bufs=2) as ystg_p,
                tc.tile_pool(name="p2row", bufs=2) as row2_p,
                tc.tile_pool(name="p2pss", bufs=4, space="PSUM") as ps_s,
                tc.tile_pool(name="p2psy", bufs=2, space="PSUM") as ps_y,
                tc.tile_pool(name="p2psl", bufs=2, space="PSUM") as ps_l,
            ):
                for h in range(H_LOC if 2 in phases else 0):
                    kv = h // (N_HEADS // N_KV_HEADS)
                    for ib in range(NIB):
                        jmax = 4 * ib + 3
                        isl = slice(ib * 512, (ib + 1) * 512)
                        y_ps = ps_y.tile([128, 512], F32, tag="y")
                        l_ps = ps_l.tile([1, 512], F32, tag="l")
                        for jt in range(jmax + 1):
                            s_ps = ps_s.tile([128, 512], F32, tag="s")
                            nc.tensor.matmul(
                                s_ps,
                                kT[:, kv, jt * 128:(jt + 1) * 128],
                                qT[:, h, isl], start=True, stop=True)
                            pt = pt_p.tile([128, 512], BF16, tag="pt")
                            nc.scalar.activation(
                                pt, s_ps, AF.Exp, scale=gscl_sb[:, h:h + 1])
                            if jt >= 4 * ib:  # diagonal j-tile: causal mask
                                ptm = pt_p.tile([128, 512], BF16, tag="ptm")
                                nc.vector.tensor_mul(
                                    ptm, pt, mask_sb[:, jt - 4 * ib, :])
                                pt = ptm
                            nc.tensor.matmul(
                                l_ps, ones_p, pt,
                                start=(jt == 0), stop=(jt == jmax))
                            nc.tensor.matmul(
                                y_ps,
                                v_sb[:, jt, kv * 128:(kv + 1) * 128],
                                pt,
                                start=(jt == 0), stop=(jt == jmax))
                        lrow = row2_p.tile([1, 512], F32R, tag="lr")
                        with nc.allow_low_precision(reason="f32r matmul operand"):
                            nc.vector.reciprocal(lrow, l_ps)
                        linv_ps = ps_l.tile([128, 512], F32, tag="l")
                        nc.tensor.matmul(linv_ps, ones_c, lrow,
                                         start=True, stop=True)
                        linv_sb = ystg_p.tile([128, 512], BF16, tag="linv")
                        nc.scalar.copy(linv_sb, linv_ps)
                        nc.vector.tensor_mul(yT[:, h, isl], y_ps, linv_sb)

            # ---------------- Phase 3: output projection ------------------
            with (
                tc.tile_pool(name="p3ow", bufs=1) as ow_p,
                tc.tile_pool(name="p3o", bufs=2) as ostg_p,
                tc.tile_pool(name="p3ps", bufs=4, space="PSUM") as ps_o,
            ):
                ow_sb = ow_p.tile([128, H_LOC, D], BF16)
                nc.sync.dma_start(ow_sb, owt_d)
                for it in range(NT if 3 in phases else 0):
                    o_sb = ostg_p.tile([128, D], F32, tag="osb")
                    for db in range(D // 512):
                        o_ps = ps_o.tile([128, 512], F32, tag="o")
                        for h in range(H_LOC):
                            nc.tensor.matmul(
                                o_ps,
                                yT[:, h, it * 128:(it + 1) * 128],
                                ow_sb[:, h, db * 512:(db + 1) * 512],
                                start=(h == 0), stop=(h == H_LOC - 1))
                        nc.scalar.copy(o_sb[:, db * 512:(db + 1) * 512], o_ps)
                    nc.sync.dma_start(out_d[it * 128:(it + 1) * 128, :], o_sb)

    nc.compile()
    return nc


def _pack_weight(w):
    """w [ncols, D]: returns [128, nh, EC, 128] with
    out[p, h, e, c] = w[h*128 + c, e*128 + p]."""
    nh = w.shape[0] // 128
    return np.ascontiguousarray(
        w.reshape(nh, 128, EC, 128).transpose(3, 0, 2, 1)).astype(BF16NP)


def make_in_maps(x, q_w, k_w, v_w, out_w, q_gain, T):
    cos, sin = _rope_tables(T)
    cosT = np.ascontiguousarray(cos.T)  # [64, T]
    sinT = np.ascontiguousarray(sin.T)
    cos2 = np.concatenate([cosT, cosT], axis=0).astype(BF16NP)  # [128, T]
    sin2 = np.concatenate([sinT, sinT], axis=0).astype(BF16NP)
    S = np.zeros((128, 128), dtype=np.float32)
    S[np.arange(64), 64 + np.arange(64)] = 1.0
    S[64 + np.arange(64), np.arange(64)] = -1.0
    swapm = np.ascontiguousarray(S.T).astype(BF16NP)

    layout = _blob_layout(T)
    in_maps = []
    for c in range(8):
        b, g = c // 2, c % 2
        # xt[p, e, t] = x[b][t, e*128+p]
        xtp = np.ascontiguousarray(
            x[b].T.reshape(EC, 128, T).transpose(1, 0, 2)).astype(BF16NP)
        # ow[p, h, d] = out_w[d, g*1024 + h*128 + p]
        owp = np.ascontiguousarray(
            out_w[:, g * 1024:(g + 1) * 1024].T
            .reshape(H_LOC, 128, D).transpose(1, 0, 2)).astype(BF16NP)
        parts = {
            "xt": xtp,
            "qwt": _pack_weight(q_w[g * 1024:(g + 1) * 1024, :]),
            "kwt": _pack_weight(k_w[g * 256:(g + 1) * 256, :]),
            "vwt": _pack_weight(v_w[g * 256:(g + 1) * 256, :]),
            "owt": owp,
            "cos2": cos2,
            "sin2": sin2,
            "swapm": swapm,
            "gains": np.ascontiguousarray(
                q_gain[g * H_LOC:(g + 1) * H_LOC]).astype(np.float32)
                .view(BF16NP),
        }
        blob = np.concatenate(
            [np.asarray(parts[name]).reshape(-1) for name, _ in layout])
        for (name, n), arr in zip(layout, [parts[n] for n, _ in layout]):
            assert np.asarray(arr).size == n, (name, np.asarray(arr).size, n)
        in_maps.append({"blob": blob})
    return in_maps


def kernel(x, q_w, k_w, v_w, out_w, q_gain, _trace=False, _trace_cores=None):
    x = np.asarray(x, dtype=np.float32)
    q_w = np.asarray(q_w, dtype=np.float32)
    k_w = np.asarray(k_w, dtype=np.float32)
    v_w = np.asarray(v_w, dtype=np.float32)
    out_w = np.asarray(out_w, dtype=np.float32)
    q_gain = np.asarray(q_gain, dtype=np.float32)
    T = x.shape[1]

    nc = build_program(T)
    in_maps = make_in_maps(x, q_w, k_w, v_w, out_w, q_gain, T)
    res = bass_utils.run_bass_kernel_spmd(
        nc, in_maps, core_ids=list(range(8)),
        trace=_trace, trace_cores=_trace_cores)
    outs = [r["out"] for r in res.results]
    full = np.stack([outs[2 * b] + outs[2 * b + 1] for b in range(B)])
    if _trace:
        return full.astype(np.float32), res
    return full.astype(np.float32)
